# revision 20
# baseline (speedup 1.0000x reference)
# Transformer-XL style relative-position attention on 8 Trainium2 NeuronCores.
#
# Contract: kernel(**inputs) takes the FULL unsharded inputs and returns the
# FULL [8, 256, 1024] output. Internally shards data-parallel over batch:
# core b computes batch element b. No collectives needed.
#
# Math (per batch element):
#   cat = [h; x]                            [512, 1024]
#   q,k,v = split(cat @ Wqkv)               heads=16, dhead=64
#   RW    = R @ Wkr                         [1024, 1024] (relative pos keys)
#   dots  = (q+u) @ k^T + rel_shift((q+v) @ RW_h^T)
#   out   = softmax(dots*8^-1 + causal/mem band mask) @ v @ Wout
#
# Key design points (v2):
#  * Valid relative offsets j - i are in [0, 256]; in rel-coordinate
#    s = j - i + 256 the window is s in [256, 512] (257 values), so only 257
#    rows of RW are ever needed (R rows 768..1023 and 0).
#  * rel_shift is a per-row shear realized through a DRAM scratch: write the
#    [128, 257] valid band of BDs = (q+v) @ RWs^T to a [128, 767] buffer
#    pre-filled with the additive mask value NEG, read it back with access
#    pattern [[766, 128], [1, 384]] (row stride 767-1) which realizes
#    band[i, j] = BDs[i, j - i + const] with mask outside the band.
#  * Attention runs over HEAD PAIRS (one 128-feature tile = 2 heads):
#    - BD and A score matmuls for the two heads are row-tiled (K=64 each,
#      partitions 0:64 / 64:128) and issued back-to-back so the PE runs them
#      concurrently in different row groups.
#    - The 4 band tiles of a pair (2 heads x 2 query blocks) go to DRAM in
#      ONE write DMA and come back in ONE SWDGE read DMA with accum_op=add,
#      which adds band+mask directly onto the A scores (term_a) in SBUF --
#      no vector-engine add needed.
#    - One wide EXP activation [128, 4*384] per pair; row sums via a single
#      DVE tensor_reduce on the 3D view; normalization as 4 tensor_scalar
#      muls split across Vector/GpSimd.
#    - Normalized attn is PE-transposed (f16 PSUM) into key-major tiles; the
#      AV matmuls are column-tiled (two heads into partition halves of one
#      PSUM tile) so the pair shares one accumulation chain.
#    - The loop is software-pipelined: pair ft's scores (BD/A/DMA) are
#      emitted before pair ft-1's exp/transpose/AV so the PE never waits on
#      the DMA+exp latency chain.
#  * All matmul operands are fp16; accumulation fp32 in PSUM.
#  * Weights are cast f32->f16 in-flight by gpsimd (SWDGE) cast-DMAs, batched
#    as quad-row-block transfers (2MB apiece) to amortize Q7 dispatch cost.
#  * The Exp activation table is preloaded at t=0 so the first attention pair
#    does not pay the ~2.7us table-load.

import numpy as np

import concourse.bass as bass
import concourse.mybir as mybir
import concourse.tile as tile
from concourse import bacc, bass_utils
from concourse.masks import make_identity
from concourse.tile import add_dep_helper
from contextlib import ExitStack

F32 = mybir.dt.float32
F16 = mybir.dt.float16
AF = mybir.ActivationFunctionType
ALU_ADD = mybir.AluOpType.add
AX_X = mybir.AxisListType.X

DIM = 1024
HEADS = 16
DHEAD = 64
B = 8
N = 256          # query tokens (x)
M = 256          # memory tokens (h)
T = M + N        # 512 keys
INNER = HEADS * DHEAD
SCALE = DHEAD ** -0.5
NEG = -30000.0   # fp16-representable; *0.125 still underflows exp
SW = 767         # BDs scratch width (relative offsets s = 1..767)
VAL0 = 255       # scratch col of first valid offset (s = 256)
NVALID = 257     # valid offsets s in [256, 512]
WIN = 384        # per-query-block live key window (3 of 4 key tiles)
NGRP = 3         # scratch groups in flight (4 buffers each)
NBUF = 4 * NGRP


def build_kernel():
    nc = bacc.Bacc("TRN2", target_bir_lowering=False, debug=False)

    x_d = nc.dram_tensor("x", [N, DIM], F32, kind="ExternalInput")
    h_d = nc.dram_tensor("h", [M, DIM], F32, kind="ExternalInput")
    wqkv_d = nc.dram_tensor("Wqkv", [DIM, 3 * INNER], F32, kind="ExternalInput")
    wkr_d = nc.dram_tensor("Wkr", [DIM, INNER], F32, kind="ExternalInput")
    r_d = nc.dram_tensor("R", [2 * T, DIM], F32, kind="ExternalInput")
    uu_d = nc.dram_tensor("uu", [128, 1], F32, kind="ExternalInput")
    vv_d = nc.dram_tensor("vv", [128, 1], F32, kind="ExternalInput")
    wout_d = nc.dram_tensor("Wout", [INNER, DIM], F32, kind="ExternalInput")
    out_d = nc.dram_tensor("out", [N, DIM], F32, kind="ExternalOutput")
    bds_d = nc.dram_tensor("bds_scratch", [NBUF, 128, SW], F16)
    junk_d = nc.dram_tensor("warm_junk", [128, 512], F16)

    with tile.TileContext(nc) as tc, ExitStack() as ctx:
        _body(ctx, tc, x_d, h_d, wqkv_d, wkr_d, r_d, uu_d, vv_d, wout_d,
              out_d, bds_d, junk_d)

    nc.compile()
    return nc


def _body(ctx, tc, x_d, h_d, wqkv_d, wkr_d, r_d, uu_d, vv_d, wout_d, out_d,
          bds_d, junk_d):
    nc = tc.nc

    const = ctx.enter_context(tc.tile_pool(name="const", bufs=1))
    persist = ctx.enter_context(tc.tile_pool(name="persist", bufs=1))
    ldpool = ctx.enter_context(tc.tile_pool(name="ld", bufs=1))
    work = ctx.enter_context(tc.tile_pool(name="work", bufs=2))
    ps_mid = ctx.enter_context(tc.tile_pool(name="ps_mid", bufs=5, space="PSUM"))
    ps_sml = ctx.enter_context(tc.tile_pool(name="ps_sml", bufs=3, space="PSUM"))

    # ---------------- PE warm-up (primes the HAM clock gate) ----------------
    junk = const.tile([128, 512], F16, tag="junk", name="junk")
    nc.vector.memset(junk, 1.0)
    pwarm = ps_mid.tile([128, 512], F32, tag="mid", name="ps_warm")
    for wi in range(16):
        nc.tensor.matmul(pwarm, junk[:, 0:128], junk,
                         start=(wi == 0), stop=(wi == 15))
    junk2 = const.tile([128, 512], F16, tag="junk2", name="junk2")
    nc.vector.tensor_copy(junk2, pwarm)

    # Preload the Exp activation table while DMAs stream (one tiny exp).
    pre = const.tile([128, 1], F32, tag="pre", name="pre")
    nc.gpsimd.memset(pre, 0.0)
    nc.scalar.activation(pre, pre, AF.Exp, bias=0.0, scale=1.0)

    # ---------------- constants ----------------
    ident = const.tile([128, 128], F32, tag="ident", name="ident")
    make_identity(nc, ident)
    ident_h = const.tile([128, 128], F16, tag="identh", name="ident_h")
    make_identity(nc, ident_h)
    ones_h = const.tile([128, 1], F16, tag="ones", name="ones_h")
    nc.vector.memset(ones_h, 1.0)
    ones_r = const.tile([1, 64], F16, tag="onesr", name="ones_r")
    nc.vector.memset(ones_r, 1.0)

    uu = const.tile([128, 1], F32, tag="uu", name="uu_sb")
    vv = const.tile([128, 1], F32, tag="vv", name="vv_sb")
    nc.sync.dma_start(out=uu, in_=uu_d[:, :])
    nc.sync.dma_start(out=vv, in_=vv_d[:, :])
    r0 = const.tile([2, DIM], F32, tag="r0", name="r0_sb")
    nc.gpsimd.memset(r0, 0.0)
    nc.sync.dma_start(out=r0[0:1, :], in_=r_d[0:1, :])

    # ---------------- activation / R loads (SWDGE cast-DMAs, queue front) ----
    # cat token order: [h (0:256) | x (256:512)]. The single SWDGE queue is
    # served in order, so acts land before the weight stream.
    catx = ldpool.tile([128, 2, DIM], F16, tag="catx", name="catx")
    cath = ldpool.tile([128, 2, DIM], F16, tag="cath", name="cath")
    r16 = ldpool.tile([128, 2, DIM], F16, tag="r16", name="r16")
    nc.gpsimd.dma_start(
        out=cath, in_=bass.AP(h_d, 0, [[DIM, 128], [128 * DIM, 2], [1, DIM]]))
    nc.gpsimd.dma_start(
        out=catx, in_=bass.AP(x_d, 0, [[DIM, 128], [128 * DIM, 2], [1, DIM]]))
    # R rows needed: offsets s=256..511 -> rows 768..1023; s=512 -> row 0
    nc.gpsimd.dma_start(
        out=r16, in_=bass.AP(r_d, 768 * DIM,
                             [[DIM, 128], [128 * DIM, 2], [1, DIM]]))

    # ---------------- weight loads (gpsimd cast-DMAs, quad row-blocks) -------
    # Wqkv [1024, 3072]: per projection 2 quads of 4 row-blocks x 1024 cols.
    def quad_load(dst_tag, dram_t, col0, ncols, nquads=2):
        tiles = []
        for qd in range(nquads):
            t_ = persist.tile([128, 4, ncols], F16, tag=f"{dst_tag}{qd}",
                              name=f"{dst_tag}{qd}")
            src = bass.AP(dram_t,
                          qd * 4 * 128 * (dram_t.shape[-1]) + col0,
                          [[dram_t.shape[-1], 128],
                           [128 * dram_t.shape[-1], 4],
                           [1, ncols]])
            tiles.append((t_, src))
        return tiles

    wq_t = quad_load("wq", wqkv_d, 0, INNER)
    wkr_t = quad_load("wkr", wkr_d, 0, INNER)
    wo_t = quad_load("wo", wout_d, 0, DIM)
    # wk as per-ft column slices and wv as 256-col chunks: the k projection
    # for head pair ft (and the val columns AV needs) become ready while the
    # rest of the weight stream is still in flight.
    wk_t = []
    for ft in range(8):
        t_ = persist.tile([128, 8, 128], F16, tag=f"wkf{ft}", name=f"wkf{ft}")
        wk_t.append((t_, bass.AP(wqkv_d, INNER + ft * 128,
                                 [[3 * INNER, 128], [128 * 3 * INNER, 8],
                                  [1, 128]])))
    wv_t = []
    for c in range(4):
        t_ = persist.tile([128, 8, 256], F16, tag=f"wvc{c}", name=f"wvc{c}")
        wv_t.append((t_, bass.AP(wqkv_d, 2 * INNER + c * 256,
                                 [[3 * INNER, 128], [128 * 3 * INNER, 8],
                                  [1, 256]])))
    # wq rides the (otherwise idle) HWDGE sync queue as f32 row-blocks and
    # is cast by the compute engines, taking 4MB off the SWDGE weight stream.
    for dt in range(8):
        wqs = ldpool.tile([128, DIM], F32, tag="wqs", name=f"wqs{dt}", bufs=2)
        nc.sync.dma_start(
            out=wqs, in_=bass.AP(wqkv_d, dt * 128 * 3 * INNER,
                                 [[3 * INNER, 128], [1, INNER]]))
        if dt % 2 == 0:
            nc.scalar.copy(wq_t[dt // 4][0][:, dt % 4], wqs)
        else:
            nc.vector.tensor_copy(wq_t[dt // 4][0][:, dt % 4], wqs)
    for t_, src in wkr_t:
        nc.gpsimd.dma_start(out=t_, in_=src)

    # Scratch mask fill: all columns the shear-read can see outside the
    # per-pair band write region stay NEG forever (writes never touch them).
    # Emitted after the wq loads so it does not delay the q projection on
    # the sync queue; the first band read is much later.
    maskw = const.tile([128, NBUF * 128], F16, tag="maskw", name="maskw")
    nc.vector.memset(maskw, NEG)
    zi1 = nc.sync.dma_start(
        out=bass.AP(bds_d, 127,
                    [[SW, 128], [128 * SW, NBUF], [1, 128]]),
        in_=maskw)
    zi2 = nc.sync.dma_start(
        out=bass.AP(bds_d, 512,
                    [[SW, 128], [128 * SW, NBUF], [1, 128]]),
        in_=maskw)
    zinit = (zi1, zi2)

    def wsl(tiles, dt):
        return tiles[dt // 4][0][:, dt % 4]

    # ---------------- transpose x, h, R ----------------
    cat16 = [cath[:, 0], cath[:, 1], catx[:, 0], catx[:, 1]]
    catT = [persist.tile([128, T], F16, tag=f"catT{dt}", name=f"catT{dt}")
            for dt in range(8)]
    for tt in range(4):
        for dt in range(8):
            tp = ps_sml.tile([128, 128], F16, tag="tp", name=f"tp_cat{tt}_{dt}")
            nc.tensor.transpose(tp, cat16[tt][:, dt * 128:(dt + 1) * 128],
                                ident_h)
            nc.vector.tensor_copy(catT[dt][:, tt * 128:(tt + 1) * 128], tp)

    NV2 = NVALID + 1  # rsubT/rwsT allocation width (col 257 unused)
    rsubT = [persist.tile([128, NV2], F16, tag=f"rsubT{dt}", name=f"rsubT{dt}")
             for dt in range(8)]
    for rt in range(2):
        for dt in range(8):
            tp = ps_sml.tile([128, 128], F16, tag="tp", name=f"tp_r{rt}_{dt}")
            nc.tensor.transpose(tp, r16[:, rt, dt * 128:(dt + 1) * 128],
                                ident_h)
            nc.scalar.copy(rsubT[dt][:, rt * 128:(rt + 1) * 128], tp)
    for dt in range(8):
        tp = ps_sml.tile([128, 2], F32, tag="tp", name=f"tp_r0_{dt}")
        nc.tensor.transpose(tp, r0[:, dt * 128:(dt + 1) * 128], ident[0:2, 0:2])
        nc.scalar.copy(rsubT[dt][:, 256:258], tp)

    # ---------------- projections ----------------
    # q_T (x tokens only) -> qu_T, qv_T [128 feat, 256 tok]
    quT = [persist.tile([128, N], F16, tag=f"quT{ft}", name=f"quT{ft}")
           for ft in range(8)]
    qvT = [persist.tile([128, N], F16, tag=f"qvT{ft}", name=f"qvT{ft}")
           for ft in range(8)]
    for ft in range(8):
        pq = ps_mid.tile([128, N], F32, tag="mid", name=f"ps_q{ft}")
        for dt in range(8):
            nc.tensor.matmul(pq, wsl(wq_t, dt)[:, ft * 128:(ft + 1) * 128],
                             catT[dt][:, M:T], start=(dt == 0), stop=(dt == 7))
        nc.vector.tensor_scalar_add(quT[ft], pq, uu)
        nc.vector.tensor_scalar_add(qvT[ft], pq, vv)

    # RWs_T[ft] = [128 feat, 257 offsets] (col 257 unused)
    rwsT = [persist.tile([128, NV2], F16, tag=f"rwsT{ft}", name=f"rwsT{ft}")
            for ft in range(8)]
    for ft in range(8):
        pr = ps_mid.tile([128, NV2], F32, tag="mid", name=f"ps_rw{ft}")
        for dt in range(8):
            nc.tensor.matmul(pr, wsl(wkr_t, dt)[:, ft * 128:(ft + 1) * 128],
                             rsubT[dt], start=(dt == 0), stop=(dt == 7))
        nc.scalar.copy(rwsT[ft], pr)

    # k loads: first two head-pair slices, then wv (AV for early pairs),
    # then the rest of wk, then wo -- all streaming on the single SWDGE queue
    # in the order the interleaved projection/attention loop consumes them.
    for t_, src in wk_t + wv_t + wo_t:
        nc.gpsimd.dma_start(out=t_, in_=src)

    # val_p[tt] = [128 tok, 16 heads x 65] -- 64 value features per head plus
    # a ones column, so the AV matmul's extra output row IS the softmax row
    # sum.
    VP = 65
    kT = [persist.tile([128, T], F16, tag=f"kT{ft}", name=f"kT{ft}")
          for ft in range(8)]
    val_p = [persist.tile([128, 16 * VP], F16, tag=f"valp{tt}",
                          name=f"valp{tt}") for tt in range(4)]
    for tt in range(4):
        nc.gpsimd.memset(
            bass.AP(val_p[tt].tensor, 64, [[16 * VP, 128], [VP, 16]]), 1.0)

    def kproj(ft):
        """k_T[ft] = [128 feat, 512 tok]"""
        pk = ps_mid.tile([128, T], F32, tag="mid", name=f"ps_k{ft}")
        for dt in range(8):
            nc.tensor.matmul(pk, wk_t[ft][0][:, dt],
                             catT[dt], start=(dt == 0), stop=(dt == 7))
        nc.scalar.copy(kT[ft], pk)

    def vproj(c):
        """val columns for head pairs 2c, 2c+1 (one 256-col wv chunk)."""
        for tt in range(4):
            pv = ps_mid.tile([128, 256], F32, tag="mid", name=f"ps_v{c}_{tt}")
            for dt in range(8):
                nc.tensor.matmul(pv, catT[dt][:, tt * 128:(tt + 1) * 128],
                                 wv_t[c][0][:, dt],
                                 start=(dt == 0), stop=(dt == 7))
            psrc = bass.AP(pv.tensor, pv.offset, [[256, 128], [64, 4], [1, 64]])
            pdst = bass.AP(val_p[tt].tensor, (c * 4) * VP,
                           [[16 * VP, 128], [VP, 4], [1, 64]])
            if tt % 2 == 0:
                nc.scalar.copy(pdst, psrc)
            else:
                nc.vector.tensor_copy(pdst, psrc)

    # ---------------- attention (head pairs, software pipelined) -------------
    attn_outT = [persist.tile([128, N], F16, tag=f"aoT{ft}", name=f"aoT{ft}")
                 for ft in range(8)]
    last_read = [None] * NGRP

    def compute_scores(ft):
        """BD matmuls, band write DMA, batched shear read DMA."""
        goff = (ft % NGRP) * 4 * 128 * SW
        bsb = work.tile([128, 4, NVALID], F16, tag="bsb", name=f"bsb{ft}", bufs=3)

        # BD = (q+v) @ RWs^T ; two heads row-tiled, issued back-to-back
        for qb in range(2):
            qsl = slice(qb * 128, (qb + 1) * 128)
            pbs = []
            for hh in range(2):
                pb = ps_mid.tile([128, NVALID], F32, tag="mid",
                                 name=f"pb{ft}_{qb}_{hh}")
                nc.tensor.matmul(pb, qvT[ft][hh * 64:(hh + 1) * 64, qsl],
                                 rwsT[ft][hh * 64:(hh + 1) * 64, 0:NVALID],
                                 start=True, stop=True)
                pbs.append(pb)
            for hh in range(2):
                kk = hh * 2 + qb
                if kk % 2 == 0:
                    nc.vector.tensor_copy(bsb[:, kk], pbs[hh])
                else:
                    nc.scalar.copy(bsb[:, kk], pbs[hh])

        w_inst = nc.sync.dma_start(
            out=bass.AP(bds_d, goff + VAL0,
                        [[SW, 128], [128 * SW, 4], [1, NVALID]]),
            in_=bsb)
        grp = ft % NGRP
        if last_read[grp] is not None:
            add_dep_helper(w_inst.ins, last_read[grp].ins, sync=True,
                           reason="scratch WAR reuse")

        band = work.tile([128, 4, WIN], F16, tag="band", name=f"band{ft}", bufs=3)
        r_inst = nc.sync.dma_start(
            out=band,
            in_=bass.AP(bds_d, goff + VAL0,
                        [[SW - 1, 128], [128 * SW, 4], [1, WIN]]))
        add_dep_helper(r_inst.ins, w_inst.ins, sync=True,
                       reason="band RAW on scratch")
        for zi in zinit:
            add_dep_helper(r_inst.ins, zi.ins, sync=True,
                           reason="band RAW on mask-init")
        last_read[grp] = r_inst

        # attnT[128 keys, jt, hh, q]; allocated + edge-zeroed here so the
        # gpsimd queue has no consume-stage work blocking the next pair.
        attnT = work.tile([128, 4, 2, N], F16, tag="attnT", name=f"attnT{ft}", bufs=3)
        nc.gpsimd.memset(attnT[:, 0, :, 128:256], 0.0)
        nc.gpsimd.memset(attnT[:, 3, :, 0:128], 0.0)
        return band, attnT

    def consume(ft, band, attnT):
        """A matmuls + band add + exp from PSUM, transpose, sums, AV."""
        expt = work.tile([128, 4, WIN], F16, tag="expt", name=f"expt{ft}",
                         bufs=2)
        for qb in range(2):
            qsl = slice(qb * 128, (qb + 1) * 128)
            pas = []
            for hh in range(2):
                pa = ps_mid.tile([128, WIN], F32, tag="mid",
                                 name=f"pa{ft}_{qb}_{hh}")
                nc.tensor.matmul(pa, quT[ft][hh * 64:(hh + 1) * 64, qsl],
                                 kT[ft][hh * 64:(hh + 1) * 64,
                                        qb * 128:qb * 128 + WIN],
                                 start=True, stop=True)
                pas.append(pa)
            for hh in range(2):
                kk = hh * 2 + qb
                nc.vector.tensor_add(pas[hh], pas[hh], band[:, kk])
                nc.scalar.activation(expt[:, kk], pas[hh], AF.Exp,
                                     bias=0.0, scale=SCALE)

        # pure transposes of the unnormalized exp into key-major layout
        nt = 0
        for qb in range(2):
            qsl = slice(qb * 128, (qb + 1) * 128)
            for w in range(3):
                jt = qb + w
                tp = ps_sml.tile([128, 2, 128], F16, tag="tp",
                                 name=f"tp_e{ft}_{qb}_{w}")
                for hh in range(2):
                    kk = hh * 2 + qb
                    nc.tensor.matmul(
                        tp[:, hh],
                        expt[:, kk][:, w * 128:(w + 1) * 128],
                        ident_h, is_transpose=True,
                        start=(hh == 0), stop=(hh == 1))
                if nt % 2 == 0:
                    nc.vector.tensor_copy(attnT[:, jt, :, qsl], tp)
                else:
                    nc.scalar.copy(attnT[:, jt, :, qsl], tp)
                nt += 1

        # AV accumulation per head; the 65th lhsT column is ones, so pav
        # row 64 = the softmax row sums (already transposed: q on free dim).
        for hh in range(2):
            h = 2 * ft + hh
            pav = ps_sml.tile([VP, N], F32, tag="tp", name=f"ps_av{ft}_{hh}")
            for jt in range(4):
                nc.tensor.matmul(pav,
                                 val_p[jt][:, h * VP:(h + 1) * VP],
                                 attnT[:, jt, hh],
                                 start=(jt == 0), stop=(jt == 3))
            srow = work.tile([1, N], F16, tag=f"srow{hh}", name=f"srow{ft}_{hh}")
            nc.scalar.copy(srow, pav[64:65, :])
            # 1/S broadcast down the 64 feature partitions: K=1 outer
            # product replicates S, then a full-width reciprocal
            srp = ps_sml.tile([64, N], F32, tag="tp", name=f"ps_sr{ft}_{hh}")
            nc.tensor.matmul(srp, ones_r, srow, start=True, stop=True)
            srec = work.tile([64, N], F32, tag=f"srec{hh}",
                             name=f"srec{ft}_{hh}")
            nc.vector.reciprocal_approx_fast(out=srec, in_=srp)
            nc.vector.tensor_mul(attn_outT[ft][hh * 64:(hh + 1) * 64, :],
                                 pav[0:64, :], srec)

    osb_p = [work.tile([128, 512], F32, tag=f"osbp{i}", name=f"osbp{i}",
                       bufs=1) for i in range(4)]

    def outproj_stage1():
        # first half of the output projection, emitted mid-attention so only
        # half the contraction remains after the last pair finishes
        for tt in range(2):
            pp = [ps_mid.tile([128, 512], F32, tag="mid",
                              name=f"ps_p{tt}_{nh}") for nh in range(2)]
            for itile in range(4):
                lhs = attn_outT[itile][:, tt * 128:(tt + 1) * 128]
                for nh in range(2):
                    nc.tensor.matmul(pp[nh], lhs,
                                     wsl(wo_t, itile)[:, nh * 512:
                                                      (nh + 1) * 512],
                                     start=(itile == 0), stop=(itile == 3))
            for nh in range(2):
                if nh == 0:
                    nc.scalar.copy(osb_p[tt * 2 + nh], pp[nh])
                else:
                    nc.vector.tensor_copy(osb_p[tt * 2 + nh], pp[nh])

    pend = []
    for ft in range(8):
        kproj(ft)
        if ft % 2 == 0:
            vproj(ft // 2)
        pend.append((ft, compute_scores(ft)))
        if len(pend) > 2:
            f0, args = pend.pop(0)
            consume(f0, *args)
            if f0 == 3:
                outproj_stage1()
    for f0, args in pend:
        consume(f0, *args)

    # ---------------- output projection (stage 2) ----------------
    for tt in range(2):
        pp = [ps_mid.tile([128, 512], F32, tag="mid", name=f"ps_o{tt}_{nh}")
              for nh in range(2)]
        for itile in range(4, 8):
            lhs = attn_outT[itile][:, tt * 128:(tt + 1) * 128]
            for nh in range(2):
                nc.tensor.matmul(pp[nh],
                                 lhs,
                                 wsl(wo_t, itile)[:, nh * 512:(nh + 1) * 512],
                                 start=(itile == 4), stop=(itile == 7))
        osb = work.tile([128, DIM], F32, tag="osb", name=f"osb{tt}", bufs=1)
        for nh in range(2):
            nc.vector.tensor_add(osb[:, nh * 512:(nh + 1) * 512],
                                 pp[nh], osb_p[tt * 2 + nh])
        nc.sync.dma_start(out=out_d[tt * 128:(tt + 1) * 128, :], in_=osb)


_NC_CACHE = {}


def _get_nc():
    if "nc" not in _NC_CACHE:
        _NC_CACHE["nc"] = build_kernel()
    return _NC_CACHE["nc"]


def _run(inputs, trace=False):
    x = np.ascontiguousarray(np.asarray(inputs["x"], dtype=np.float32))
    h = np.ascontiguousarray(np.asarray(inputs["h"], dtype=np.float32))
    wqkv = np.ascontiguousarray(np.asarray(inputs["Wqkv"], dtype=np.float32))
    wkr = np.ascontiguousarray(np.asarray(inputs["Wkr"], dtype=np.float32))
    r = np.ascontiguousarray(np.asarray(inputs["R"], dtype=np.float32))
    u = np.asarray(inputs["u"], dtype=np.float32)
    v = np.asarray(inputs["v"], dtype=np.float32)
    wout = np.ascontiguousarray(np.asarray(inputs["Wout"], dtype=np.float32))
    uu = np.ascontiguousarray(np.tile(u, 2).reshape(128, 1))
    vv = np.ascontiguousarray(np.tile(v, 2).reshape(128, 1))

    nc = _get_nc()
    in_maps = [
        {"x": x[b], "h": h[b], "Wqkv": wqkv, "Wkr": wkr, "R": r,
         "uu": uu, "vv": vv, "Wout": wout}
        for b in range(B)
    ]
    res = bass_utils.run_bass_kernel_spmd(
        nc, in_maps, core_ids=list(range(B)), trace=trace)
    out = np.stack([res.results[b]["out"] for b in range(B)])
    return out.astype(np.float32), res


def kernel(**inputs):
    out, _ = _run(inputs, trace=False)
    return out


# revision 21
# speedup vs baseline: 1.2376x; 1.2376x over previous
# Transformer-XL style relative-position attention on 8 Trainium2 NeuronCores.
#
# Contract: kernel(**inputs) takes the FULL unsharded inputs and returns the
# FULL [8, 256, 1024] output. Internally shards data-parallel over batch:
# core b computes batch element b. No collectives needed.
#
# Math (per batch element):
#   cat = [h; x]                            [512, 1024]
#   q,k,v = split(cat @ Wqkv)               heads=16, dhead=64
#   RW    = R @ Wkr                         [1024, 1024] (relative pos keys)
#   dots  = (q+u) @ k^T + rel_shift((q+v) @ RW_h^T)
#   out   = softmax(dots*8^-1 + causal/mem band mask) @ v @ Wout
#
# Key design points (v2):
#  * Valid relative offsets j - i are in [0, 256]; in rel-coordinate
#    s = j - i + 256 the window is s in [256, 512] (257 values), so only 257
#    rows of RW are ever needed (R rows 768..1023 and 0).
#  * rel_shift is a per-row shear realized through a DRAM scratch: write the
#    [128, 257] valid band of BDs = (q+v) @ RWs^T to a [128, 767] buffer
#    pre-filled with the additive mask value NEG, read it back with access
#    pattern [[766, 128], [1, 384]] (row stride 767-1) which realizes
#    band[i, j] = BDs[i, j - i + const] with mask outside the band.
#  * Attention runs over HEAD PAIRS (one 128-feature tile = 2 heads):
#    - BD and A score matmuls for the two heads are row-tiled (K=64 each,
#      partitions 0:64 / 64:128) and issued back-to-back so the PE runs them
#      concurrently in different row groups.
#    - The 4 band tiles of a pair (2 heads x 2 query blocks) go to DRAM in
#      ONE write DMA and come back in ONE SWDGE read DMA with accum_op=add,
#      which adds band+mask directly onto the A scores (term_a) in SBUF --
#      no vector-engine add needed.
#    - One wide EXP activation [128, 4*384] per pair; row sums via a single
#      DVE tensor_reduce on the 3D view; normalization as 4 tensor_scalar
#      muls split across Vector/GpSimd.
#    - Normalized attn is PE-transposed (f16 PSUM) into key-major tiles; the
#      AV matmuls are column-tiled (two heads into partition halves of one
#      PSUM tile) so the pair shares one accumulation chain.
#    - The loop is software-pipelined: pair ft's scores (BD/A/DMA) are
#      emitted before pair ft-1's exp/transpose/AV so the PE never waits on
#      the DMA+exp latency chain.
#  * All matmul operands are fp16; accumulation fp32 in PSUM.
#  * Weights are cast f32->f16 in-flight by gpsimd (SWDGE) cast-DMAs, batched
#    as quad-row-block transfers (2MB apiece) to amortize Q7 dispatch cost.
#  * The Exp activation table is preloaded at t=0 so the first attention pair
#    does not pay the ~2.7us table-load.

import numpy as np

import concourse.bass as bass
import concourse.mybir as mybir
import concourse.tile as tile
from concourse import bacc, bass_utils
from concourse.masks import make_identity
from concourse.tile import add_dep_helper
from contextlib import ExitStack

F32 = mybir.dt.float32
F16 = mybir.dt.float16
AF = mybir.ActivationFunctionType
ALU_ADD = mybir.AluOpType.add
AX_X = mybir.AxisListType.X

DIM = 1024
HEADS = 16
DHEAD = 64
B = 8
N = 256          # query tokens (x)
M = 256          # memory tokens (h)
T = M + N        # 512 keys
INNER = HEADS * DHEAD
SCALE = DHEAD ** -0.5
NEG = -30000.0   # fp16-representable; *0.125 still underflows exp
SW = 767         # BDs scratch width (relative offsets s = 1..767)
VAL0 = 255       # scratch col of first valid offset (s = 256)
NVALID = 257     # valid offsets s in [256, 512]
WIN = 384        # per-query-block live key window (3 of 4 key tiles)
NGRP = 3         # scratch groups in flight (4 buffers each)
NBUF = 4 * NGRP


def build_kernel():
    nc = bacc.Bacc("TRN2", target_bir_lowering=False, debug=False)

    x_d = nc.dram_tensor("x", [N, DIM], F32, kind="ExternalInput")
    h_d = nc.dram_tensor("h", [M, DIM], F32, kind="ExternalInput")
    wqkv_d = nc.dram_tensor("Wqkv", [DIM, 3 * INNER], F32, kind="ExternalInput")
    wkr_d = nc.dram_tensor("Wkr", [DIM, INNER], F32, kind="ExternalInput")
    r_d = nc.dram_tensor("R", [2 * T, DIM], F32, kind="ExternalInput")
    uu_d = nc.dram_tensor("uu", [128, 1], F32, kind="ExternalInput")
    vv_d = nc.dram_tensor("vv", [128, 1], F32, kind="ExternalInput")
    wout_d = nc.dram_tensor("Wout", [INNER, DIM], F32, kind="ExternalInput")
    out_d = nc.dram_tensor("out", [N, DIM], F32, kind="ExternalOutput")
    bds_d = nc.dram_tensor("bds_scratch", [NBUF, 128, SW], F16)
    junk_d = nc.dram_tensor("warm_junk", [128, 512], F16)

    with tile.TileContext(nc) as tc, ExitStack() as ctx:
        _body(ctx, tc, x_d, h_d, wqkv_d, wkr_d, r_d, uu_d, vv_d, wout_d,
              out_d, bds_d, junk_d)

    nc.compile()
    return nc


def _body(ctx, tc, x_d, h_d, wqkv_d, wkr_d, r_d, uu_d, vv_d, wout_d, out_d,
          bds_d, junk_d):
    nc = tc.nc

    const = ctx.enter_context(tc.tile_pool(name="const", bufs=1))
    persist = ctx.enter_context(tc.tile_pool(name="persist", bufs=1))
    ldpool = ctx.enter_context(tc.tile_pool(name="ld", bufs=1))
    work = ctx.enter_context(tc.tile_pool(name="work", bufs=2))
    ps_mid = ctx.enter_context(tc.tile_pool(name="ps_mid", bufs=5, space="PSUM"))
    ps_sml = ctx.enter_context(tc.tile_pool(name="ps_sml", bufs=3, space="PSUM"))

    # ---------------- PE warm-up (primes the HAM clock gate) ----------------
    junk = const.tile([128, 512], F16, tag="junk", name="junk")
    nc.vector.memset(junk, 1.0)
    pwarm = ps_mid.tile([128, 512], F32, tag="mid", name="ps_warm")
    for wi in range(16):
        nc.tensor.matmul(pwarm, junk[:, 0:128], junk,
                         start=(wi == 0), stop=(wi == 15))
    junk2 = const.tile([128, 512], F16, tag="junk2", name="junk2")
    nc.vector.tensor_copy(junk2, pwarm)

    # Preload the Exp activation table while DMAs stream (one tiny exp).
    pre = const.tile([128, 1], F32, tag="pre", name="pre")
    nc.gpsimd.memset(pre, 0.0)
    nc.scalar.activation(pre, pre, AF.Exp, bias=0.0, scale=1.0)

    # ---------------- constants ----------------
    ident = const.tile([128, 128], F32, tag="ident", name="ident")
    make_identity(nc, ident)
    ident_h = const.tile([128, 128], F16, tag="identh", name="ident_h")
    make_identity(nc, ident_h)
    ones_h = const.tile([128, 1], F16, tag="ones", name="ones_h")
    nc.vector.memset(ones_h, 1.0)
    ones_r = const.tile([1, 64], F16, tag="onesr", name="ones_r")
    nc.vector.memset(ones_r, 1.0)

    uu = const.tile([128, 1], F32, tag="uu", name="uu_sb")
    vv = const.tile([128, 1], F32, tag="vv", name="vv_sb")
    nc.sync.dma_start(out=uu, in_=uu_d[:, :])
    nc.sync.dma_start(out=vv, in_=vv_d[:, :])
    r0 = const.tile([2, DIM], F32, tag="r0", name="r0_sb")
    nc.gpsimd.memset(r0, 0.0)
    nc.sync.dma_start(out=r0[0:1, :], in_=r_d[0:1, :])

    # ---------------- activation / R loads (SWDGE cast-DMAs, queue front) ----
    # cat token order: [h (0:256) | x (256:512)]. The single SWDGE queue is
    # served in order, so acts land before the weight stream.
    catx = ldpool.tile([128, 2, DIM], F16, tag="catx", name="catx")
    cath = ldpool.tile([128, 2, DIM], F16, tag="cath", name="cath")
    r16 = ldpool.tile([128, 2, DIM], F16, tag="r16", name="r16")
    nc.gpsimd.dma_start(
        out=cath, in_=bass.AP(h_d, 0, [[DIM, 128], [128 * DIM, 2], [1, DIM]]))
    nc.gpsimd.dma_start(
        out=catx, in_=bass.AP(x_d, 0, [[DIM, 128], [128 * DIM, 2], [1, DIM]]))
    # R rows needed: offsets s=256..511 -> rows 768..1023; s=512 -> row 0
    nc.gpsimd.dma_start(
        out=r16, in_=bass.AP(r_d, 768 * DIM,
                             [[DIM, 128], [128 * DIM, 2], [1, DIM]]))

    # ---------------- weight loads (gpsimd cast-DMAs, quad row-blocks) -------
    # Wqkv [1024, 3072]: per projection 2 quads of 4 row-blocks x 1024 cols.
    def quad_load(dst_tag, dram_t, col0, ncols, nquads=2):
        tiles = []
        for qd in range(nquads):
            t_ = persist.tile([128, 4, ncols], F16, tag=f"{dst_tag}{qd}",
                              name=f"{dst_tag}{qd}")
            src = bass.AP(dram_t,
                          qd * 4 * 128 * (dram_t.shape[-1]) + col0,
                          [[dram_t.shape[-1], 128],
                           [128 * dram_t.shape[-1], 4],
                           [1, ncols]])
            tiles.append((t_, src))
        return tiles

    wq_t = quad_load("wq", wqkv_d, 0, INNER)
    wkr_t = quad_load("wkr", wkr_d, 0, INNER)
    wo_t = quad_load("wo", wout_d, 0, DIM)
    # wk as per-ft column slices and wv as 256-col chunks: the k projection
    # for head pair ft (and the val columns AV needs) become ready while the
    # rest of the weight stream is still in flight.
    wk_t = []
    for ft in range(8):
        t_ = persist.tile([128, 8, 128], F16, tag=f"wkf{ft}", name=f"wkf{ft}")
        wk_t.append((t_, bass.AP(wqkv_d, INNER + ft * 128,
                                 [[3 * INNER, 128], [128 * 3 * INNER, 8],
                                  [1, 128]])))
    wv_t = []
    for c in range(4):
        t_ = persist.tile([128, 8, 256], F16, tag=f"wvc{c}", name=f"wvc{c}")
        wv_t.append((t_, bass.AP(wqkv_d, 2 * INNER + c * 256,
                                 [[3 * INNER, 128], [128 * 3 * INNER, 8],
                                  [1, 256]])))
    for t_, src in wq_t + wkr_t:
        nc.gpsimd.dma_start(out=t_, in_=src)

    # Scratch mask fill: all columns the shear-read can see outside the
    # per-pair band write region stay NEG forever (writes never touch them).
    # Emitted after the wq loads so it does not delay the q projection on
    # the sync queue; the first band read is much later.
    maskw = const.tile([128, NBUF * 128], F16, tag="maskw", name="maskw")
    nc.vector.memset(maskw, NEG)
    zi1 = nc.sync.dma_start(
        out=bass.AP(bds_d, 127,
                    [[SW, 128], [128 * SW, NBUF], [1, 128]]),
        in_=maskw)
    zi2 = nc.sync.dma_start(
        out=bass.AP(bds_d, 512,
                    [[SW, 128], [128 * SW, NBUF], [1, 128]]),
        in_=maskw)
    zinit = (zi1, zi2)

    def wsl(tiles, dt):
        return tiles[dt // 4][0][:, dt % 4]

    # ---------------- transpose x, h, R ----------------
    cat16 = [cath[:, 0], cath[:, 1], catx[:, 0], catx[:, 1]]
    catT = [persist.tile([128, T], F16, tag=f"catT{dt}", name=f"catT{dt}")
            for dt in range(8)]
    for tt in range(4):
        for dt in range(8):
            tp = ps_sml.tile([128, 128], F16, tag="tp", name=f"tp_cat{tt}_{dt}")
            nc.tensor.transpose(tp, cat16[tt][:, dt * 128:(dt + 1) * 128],
                                ident_h)
            nc.vector.tensor_copy(catT[dt][:, tt * 128:(tt + 1) * 128], tp)

    NV2 = NVALID + 1  # rsubT/rwsT allocation width (col 257 unused)
    rsubT = [persist.tile([128, NV2], F16, tag=f"rsubT{dt}", name=f"rsubT{dt}")
             for dt in range(8)]
    for rt in range(2):
        for dt in range(8):
            tp = ps_sml.tile([128, 128], F16, tag="tp", name=f"tp_r{rt}_{dt}")
            nc.tensor.transpose(tp, r16[:, rt, dt * 128:(dt + 1) * 128],
                                ident_h)
            nc.scalar.copy(rsubT[dt][:, rt * 128:(rt + 1) * 128], tp)
    for dt in range(8):
        tp = ps_sml.tile([128, 2], F32, tag="tp", name=f"tp_r0_{dt}")
        nc.tensor.transpose(tp, r0[:, dt * 128:(dt + 1) * 128], ident[0:2, 0:2])
        nc.scalar.copy(rsubT[dt][:, 256:258], tp)

    # ---------------- projections ----------------
    # q_T (x tokens only) -> qu_T, qv_T [128 feat, 256 tok]
    quT = [persist.tile([128, N], F16, tag=f"quT{ft}", name=f"quT{ft}")
           for ft in range(8)]
    qvT = [persist.tile([128, N], F16, tag=f"qvT{ft}", name=f"qvT{ft}")
           for ft in range(8)]
    for ft in range(8):
        pq = ps_mid.tile([128, N], F32, tag="mid", name=f"ps_q{ft}")
        for dt in range(8):
            nc.tensor.matmul(pq, wsl(wq_t, dt)[:, ft * 128:(ft + 1) * 128],
                             catT[dt][:, M:T], start=(dt == 0), stop=(dt == 7))
        nc.vector.tensor_scalar_add(quT[ft], pq, uu)
        nc.vector.tensor_scalar_add(qvT[ft], pq, vv)

    # RWs_T[ft] = [128 feat, 257 offsets] (col 257 unused)
    rwsT = [persist.tile([128, NV2], F16, tag=f"rwsT{ft}", name=f"rwsT{ft}")
            for ft in range(8)]
    for ft in range(8):
        pr = ps_mid.tile([128, NV2], F32, tag="mid", name=f"ps_rw{ft}")
        for dt in range(8):
            nc.tensor.matmul(pr, wsl(wkr_t, dt)[:, ft * 128:(ft + 1) * 128],
                             rsubT[dt], start=(dt == 0), stop=(dt == 7))
        nc.scalar.copy(rwsT[ft], pr)

    # k loads: first two head-pair slices, then wv (AV for early pairs),
    # then the rest of wk, then wo -- all streaming on the single SWDGE queue
    # in the order the interleaved projection/attention loop consumes them.
    for t_, src in wk_t + wv_t + wo_t:
        nc.gpsimd.dma_start(out=t_, in_=src)

    # val_p[tt] = [128 tok, 16 heads x 65] -- 64 value features per head plus
    # a ones column, so the AV matmul's extra output row IS the softmax row
    # sum.
    VP = 65
    kT = [persist.tile([128, T], F16, tag=f"kT{ft}", name=f"kT{ft}")
          for ft in range(8)]
    val_p = [persist.tile([128, 16 * VP], F16, tag=f"valp{tt}",
                          name=f"valp{tt}") for tt in range(4)]
    for tt in range(4):
        nc.gpsimd.memset(
            bass.AP(val_p[tt].tensor, 64, [[16 * VP, 128], [VP, 16]]), 1.0)

    def kproj(ft):
        """k_T[ft] = [128 feat, 512 tok]"""
        pk = ps_mid.tile([128, T], F32, tag="mid", name=f"ps_k{ft}")
        for dt in range(8):
            nc.tensor.matmul(pk, wk_t[ft][0][:, dt],
                             catT[dt], start=(dt == 0), stop=(dt == 7))
        nc.scalar.copy(kT[ft], pk)

    def vproj(c):
        """val columns for head pairs 2c, 2c+1 (one 256-col wv chunk)."""
        for tt in range(4):
            pv = ps_mid.tile([128, 256], F32, tag="mid", name=f"ps_v{c}_{tt}")
            for dt in range(8):
                nc.tensor.matmul(pv, catT[dt][:, tt * 128:(tt + 1) * 128],
                                 wv_t[c][0][:, dt],
                                 start=(dt == 0), stop=(dt == 7))
            psrc = bass.AP(pv.tensor, pv.offset, [[256, 128], [64, 4], [1, 64]])
            pdst = bass.AP(val_p[tt].tensor, (c * 4) * VP,
                           [[16 * VP, 128], [VP, 4], [1, 64]])
            if tt % 2 == 0:
                nc.scalar.copy(pdst, psrc)
            else:
                nc.vector.tensor_copy(pdst, psrc)

    # ---------------- attention (head pairs, software pipelined) -------------
    attn_outT = [persist.tile([128, N], F16, tag=f"aoT{ft}", name=f"aoT{ft}")
                 for ft in range(8)]
    last_read = [None] * NGRP

    def compute_scores(ft):
        """BD matmuls, band write DMA, batched shear read DMA."""
        goff = (ft % NGRP) * 4 * 128 * SW
        bsb = work.tile([128, 4, NVALID], F16, tag="bsb", name=f"bsb{ft}", bufs=3)

        # BD = (q+v) @ RWs^T ; two heads row-tiled, issued back-to-back
        for qb in range(2):
            qsl = slice(qb * 128, (qb + 1) * 128)
            pbs = []
            for hh in range(2):
                pb = ps_mid.tile([128, NVALID], F32, tag="mid",
                                 name=f"pb{ft}_{qb}_{hh}")
                nc.tensor.matmul(pb, qvT[ft][hh * 64:(hh + 1) * 64, qsl],
                                 rwsT[ft][hh * 64:(hh + 1) * 64, 0:NVALID],
                                 start=True, stop=True)
                pbs.append(pb)
            for hh in range(2):
                kk = hh * 2 + qb
                if kk % 2 == 0:
                    nc.vector.tensor_copy(bsb[:, kk], pbs[hh])
                else:
                    nc.scalar.copy(bsb[:, kk], pbs[hh])

        w_inst = nc.sync.dma_start(
            out=bass.AP(bds_d, goff + VAL0,
                        [[SW, 128], [128 * SW, 4], [1, NVALID]]),
            in_=bsb)
        grp = ft % NGRP
        if last_read[grp] is not None:
            add_dep_helper(w_inst.ins, last_read[grp].ins, sync=True,
                           reason="scratch WAR reuse")

        band = work.tile([128, 4, WIN], F16, tag="band", name=f"band{ft}", bufs=3)
        r_inst = nc.sync.dma_start(
            out=band,
            in_=bass.AP(bds_d, goff + VAL0,
                        [[SW - 1, 128], [128 * SW, 4], [1, WIN]]))
        add_dep_helper(r_inst.ins, w_inst.ins, sync=True,
                       reason="band RAW on scratch")
        for zi in zinit:
            add_dep_helper(r_inst.ins, zi.ins, sync=True,
                           reason="band RAW on mask-init")
        last_read[grp] = r_inst

        # attnT[128 keys, jt, hh, q]; allocated + edge-zeroed here so the
        # gpsimd queue has no consume-stage work blocking the next pair.
        attnT = work.tile([128, 4, 2, N], F16, tag="attnT", name=f"attnT{ft}", bufs=3)
        nc.gpsimd.memset(attnT[:, 0, :, 128:256], 0.0)
        nc.gpsimd.memset(attnT[:, 3, :, 0:128], 0.0)
        return band, attnT

    def consume(ft, band, attnT):
        """A matmuls + band add + exp from PSUM, transpose, sums, AV."""
        expt = work.tile([128, 4, WIN], F16, tag="expt", name=f"expt{ft}",
                         bufs=2)
        for qb in range(2):
            qsl = slice(qb * 128, (qb + 1) * 128)
            pas = []
            for hh in range(2):
                pa = ps_mid.tile([128, WIN], F32, tag="mid",
                                 name=f"pa{ft}_{qb}_{hh}")
                nc.tensor.matmul(pa, quT[ft][hh * 64:(hh + 1) * 64, qsl],
                                 kT[ft][hh * 64:(hh + 1) * 64,
                                        qb * 128:qb * 128 + WIN],
                                 start=True, stop=True)
                pas.append(pa)
            for hh in range(2):
                kk = hh * 2 + qb
                nc.vector.tensor_add(pas[hh], pas[hh], band[:, kk])
                nc.scalar.activation(expt[:, kk], pas[hh], AF.Exp,
                                     bias=0.0, scale=SCALE)

        # pure transposes of the unnormalized exp into key-major layout
        nt = 0
        for qb in range(2):
            qsl = slice(qb * 128, (qb + 1) * 128)
            for w in range(3):
                jt = qb + w
                tp = ps_sml.tile([128, 2, 128], F16, tag="tp",
                                 name=f"tp_e{ft}_{qb}_{w}")
                for hh in range(2):
                    kk = hh * 2 + qb
                    nc.tensor.matmul(
                        tp[:, hh],
                        expt[:, kk][:, w * 128:(w + 1) * 128],
                        ident_h, is_transpose=True,
                        start=(hh == 0), stop=(hh == 1))
                if nt % 2 == 0:
                    nc.vector.tensor_copy(attnT[:, jt, :, qsl], tp)
                else:
                    nc.scalar.copy(attnT[:, jt, :, qsl], tp)
                nt += 1

        # AV accumulation per head; the 65th lhsT column is ones, so pav
        # row 64 = the softmax row sums (already transposed: q on free dim).
        for hh in range(2):
            h = 2 * ft + hh
            pav = ps_sml.tile([VP, N], F32, tag="tp", name=f"ps_av{ft}_{hh}")
            for jt in range(4):
                nc.tensor.matmul(pav,
                                 val_p[jt][:, h * VP:(h + 1) * VP],
                                 attnT[:, jt, hh],
                                 start=(jt == 0), stop=(jt == 3))
            srow = work.tile([1, N], F16, tag=f"srow{hh}", name=f"srow{ft}_{hh}")
            nc.scalar.copy(srow, pav[64:65, :])
            # 1/S broadcast down the 64 feature partitions: K=1 outer
            # product replicates S, then a full-width reciprocal
            srp = ps_sml.tile([64, N], F32, tag="tp", name=f"ps_sr{ft}_{hh}")
            nc.tensor.matmul(srp, ones_r, srow, start=True, stop=True)
            srec = work.tile([64, N], F32, tag=f"srec{hh}",
                             name=f"srec{ft}_{hh}")
            nc.vector.reciprocal_approx_fast(out=srec, in_=srp)
            nc.vector.tensor_mul(attn_outT[ft][hh * 64:(hh + 1) * 64, :],
                                 pav[0:64, :], srec)

    osb_p = [work.tile([128, 512], F32, tag=f"osbp{i}", name=f"osbp{i}",
                       bufs=1) for i in range(4)]

    def outproj_stage1():
        # first half of the output projection, emitted mid-attention so only
        # half the contraction remains after the last pair finishes
        for tt in range(2):
            pp = [ps_mid.tile([128, 512], F32, tag="mid",
                              name=f"ps_p{tt}_{nh}") for nh in range(2)]
            for itile in range(4):
                lhs = attn_outT[itile][:, tt * 128:(tt + 1) * 128]
                for nh in range(2):
                    nc.tensor.matmul(pp[nh], lhs,
                                     wsl(wo_t, itile)[:, nh * 512:
                                                      (nh + 1) * 512],
                                     start=(itile == 0), stop=(itile == 3))
            for nh in range(2):
                if nh == 0:
                    nc.scalar.copy(osb_p[tt * 2 + nh], pp[nh])
                else:
                    nc.vector.tensor_copy(osb_p[tt * 2 + nh], pp[nh])

    pend = []
    for ft in range(8):
        kproj(ft)
        if ft % 2 == 0:
            vproj(ft // 2)
        pend.append((ft, compute_scores(ft)))
        if len(pend) > 2:
            f0, args = pend.pop(0)
            consume(f0, *args)
            if f0 == 3:
                outproj_stage1()
    for f0, args in pend:
        consume(f0, *args)

    # ---------------- output projection (stage 2) ----------------
    for tt in range(2):
        pp = [ps_mid.tile([128, 512], F32, tag="mid", name=f"ps_o{tt}_{nh}")
              for nh in range(2)]
        for itile in range(4, 8):
            lhs = attn_outT[itile][:, tt * 128:(tt + 1) * 128]
            for nh in range(2):
                nc.tensor.matmul(pp[nh],
                                 lhs,
                                 wsl(wo_t, itile)[:, nh * 512:(nh + 1) * 512],
                                 start=(itile == 4), stop=(itile == 7))
        osb = work.tile([128, DIM], F32, tag="osb", name=f"osb{tt}", bufs=1)
        for nh in range(2):
            nc.vector.tensor_add(osb[:, nh * 512:(nh + 1) * 512],
                                 pp[nh], osb_p[tt * 2 + nh])
        nc.sync.dma_start(out=out_d[tt * 128:(tt + 1) * 128, :], in_=osb)


_NC_CACHE = {}


def _get_nc():
    if "nc" not in _NC_CACHE:
        _NC_CACHE["nc"] = build_kernel()
    return _NC_CACHE["nc"]


def _run(inputs, trace=False):
    x = np.ascontiguousarray(np.asarray(inputs["x"], dtype=np.float32))
    h = np.ascontiguousarray(np.asarray(inputs["h"], dtype=np.float32))
    wqkv = np.ascontiguousarray(np.asarray(inputs["Wqkv"], dtype=np.float32))
    wkr = np.ascontiguousarray(np.asarray(inputs["Wkr"], dtype=np.float32))
    r = np.ascontiguousarray(np.asarray(inputs["R"], dtype=np.float32))
    u = np.asarray(inputs["u"], dtype=np.float32)
    v = np.asarray(inputs["v"], dtype=np.float32)
    wout = np.ascontiguousarray(np.asarray(inputs["Wout"], dtype=np.float32))
    uu = np.ascontiguousarray(np.tile(u, 2).reshape(128, 1))
    vv = np.ascontiguousarray(np.tile(v, 2).reshape(128, 1))

    nc = _get_nc()
    in_maps = [
        {"x": x[b], "h": h[b], "Wqkv": wqkv, "Wkr": wkr, "R": r,
         "uu": uu, "vv": vv, "Wout": wout}
        for b in range(B)
    ]
    res = bass_utils.run_bass_kernel_spmd(
        nc, in_maps, core_ids=list(range(B)), trace=trace)
    out = np.stack([res.results[b]["out"] for b in range(B)])
    return out.astype(np.float32), res


def kernel(**inputs):
    out, _ = _run(inputs, trace=False)
    return out
